# revision 33
# baseline (speedup 1.0000x reference)
import os
import sys

sys.path.insert(0, "/opt/trn_rl_repo")

import numpy as np
import ml_dtypes

import concourse.bass as bass
import concourse.bacc as bacc
import concourse.mybir as mybir
from concourse import masks
from concourse.bass_utils import run_bass_kernel_spmd
from concourse.tile import TileContext

S = 1024
DIM = 2560
HD = 128
NH = 20
NKV = 5
GS = 128
THETA = 500000.0
EPS = 1e-05
KBASE = NH * HD            # k rows start in w_qkv
VBASE = KBASE + NKV * HD   # v rows start
NC = 8
KCH = DIM // 128           # 20 k-chunks
WQCOLS = 7 * 128           # [qs0 qs1 qs2 kA vA kB vB]
OC = DIM // NC             # 320 output cols per core
MT = S // 128              # 8 token tiles
NQ = 4                     # attention/o_proj processed in 256-token quarters
QW = S // NQ               # 256

# head assignment per core: [slot0, slot1, slot2]; None = garbage slot
HEADS = [
    [0, 1, 8], [2, 3, 9], [4, 5, 10], [6, 7, 11],
    [12, 13, None], [14, 15, None], [16, 17, None], [18, 19, None],
]
GA = [0, 0, 1, 1, 3, 3, 4, 4]              # kv group for slots 0,1
GB = [2, 2, 2, 2, None, None, None, None]  # kv group for slot 2
REAL_CHUNKS = [j * 3 + s for j in range(NC) for s in range(3) if HEADS[j][s] is not None]
assert len(REAL_CHUNKS) == NH

FP16 = np.float16
SCALE = float(HD) ** -0.5
ESHIFT = -2.0  # exp(score*SCALE + ESHIFT); cancels in softmax ratio

_cached = {}


def _build_nc():
    nc = bacc.Bacc("TRN2", target_bir_lowering=False, debug=False, num_devices=NC)
    f32 = mybir.dt.float32
    f16 = mybir.dt.float16
    i16 = mybir.dt.int16

    x_d = nc.declare_dram_parameter("x", [S, DIM], f32, isOutput=False)
    # weights/tables pre-arranged on host into [128, ...] partition-major
    wq_d = nc.declare_dram_parameter("wq", [128, 4 * 5 * WQCOLS], f16, isOutput=False)
    wo_d = nc.declare_dram_parameter("wo", [128, NH * OC], f16, isOutput=False)
    tq1_d = nc.declare_dram_parameter("tq1", [128, MT * HD], f32, isOutput=False)
    tq2_d = nc.declare_dram_parameter("tq2", [128, MT * HD], f32, isOutput=False)
    tk1_d = nc.declare_dram_parameter("tk1", [128, MT * HD], f32, isOutput=False)
    tk2_d = nc.declare_dram_parameter("tk2", [128, MT * HD], f32, isOutput=False)
    # causal mask for the diagonal k-pair of each 256-wide q group:
    # [tri | ones | zeros | tri] over (kc_bit, q_tile)
    cmask_d = nc.declare_dram_parameter("cmask", [128, 512], f16, isOutput=False)
    # output stored transposed [OC, S]; host transposes back
    out_d = nc.declare_dram_parameter("out", [OC, S], f32, isOutput=True)

    agin_d = [nc.dram_tensor(f"agin{t}", [384, QW], f16, kind="Internal")
              for t in range(NQ)]
    agout_d = [nc.dram_tensor(f"agout{t}", [NC * 384, QW], f16, kind="Internal",
                              addr_space="Shared") for t in range(NQ)]

    from contextlib import ExitStack
    with TileContext(nc) as tc:
        with ExitStack() as stack:
            pool = lambda *a, **k: stack.enter_context(tc.tile_pool(*a, **k))
            cst = pool(name="cst", bufs=1)
            wqp = pool(name="wqp", bufs=4)
            wop = pool(name="wop", bufs=1)
            kvp = pool(name="kvp", bufs=1)
            xap = pool(name="xap", bufs=3)
            q16p = pool(name="q16p", bufs=2)
            qfp = pool(name="qfp", bufs=2)
            qmp = pool(name="qmp", bufs=2)
            rbp = pool(name="rbp", bufs=2)
            nrp = pool(name="nrp", bufs=2)
            ptp = pool(name="ptp", bufs=6)
            atp = pool(name="atp", bufs=2)
            agtp = pool(name="agtp", bufs=3)
            oevp = pool(name="oevp", bufs=2)
            # PSUM pools (banks): tr 2 + cs 1 + sc 2 + av 1 + cmm 2 = 8
            trp = pool(name="trp", bufs=2, space="PSUM")
            csp = pool(name="csp", bufs=1, space="PSUM")
            scp = pool(name="scp", bufs=2, space="PSUM")
            avpp = pool(name="avp", bufs=1, space="PSUM")
            cmm = pool(name="cmm", bufs=2, space="PSUM")
            ident_h = cst.tile([128, 128], f16, tag="idh", name="idh")
            masks.make_identity(nc, ident_h[:, :])
            ones_c16 = cst.tile([128, 1], f16, tag="oc16", name="oc16")
            nc.vector.memset(ones_c16[:, :], 1.0)
            ones_r32 = cst.tile([1, 128], f32, tag="or32", name="or32")
            nc.vector.memset(ones_r32[:, :], 1.0)
            eshift = cst.tile([128, 1], f32, tag="esh", name="esh")
            nc.vector.memset(eshift[:, :], ESHIFT)
            cmask = cst.tile([128, 512], f16, tag="cm", name="cm")
            nc.sync.dma_start(out=cmask[:, :], in_=cmask_d[:, :])

            s_cols = cst.tile([128, MT], f32, tag="scols", name="scols")
            rs_cols = cst.tile([128, MT], f32, tag="rscols", name="rscols")

            xa_tiles = {}

            def emit_xload(m):
                xa = xap.tile([128, DIM], f32, tag="x", name="x")
                nc.sync.dma_start(out=xa[:, :], in_=x_d[m * 128:(m + 1) * 128, :])
                mx = xap.tile([128, 1], f32, tag="mx", name="mx")
                nc.vector.tensor_reduce(mx[:, :], xa[:, :], mybir.AxisListType.X,
                                        mybir.AluOpType.max,
                                        apply_absolute_value=True)
                mx2 = xap.tile([128, 1], f32, tag="mx2", name="mx2")
                nc.vector.tensor_scalar_max(mx2[:, :], mx[:, :], 1e-5)
                nc.vector.tensor_scalar_mul(rs_cols[:, m:m + 1], mx2[:, :],
                                            1.0 / 127.0)
                nc.vector.reciprocal_approx_fast(s_cols[:, m:m + 1],
                                                 rs_cols[:, m:m + 1])
                xa_tiles[m] = xa

            emit_xload(0)
            emit_xload(1)

            tabs = {}
            for nm, d in (("tq1", tq1_d), ("tq2", tq2_d),
                          ("tk1", tk1_d), ("tk2", tk2_d)):
                t = cst.tile([128, MT, HD], f32, tag=f"tb{nm}", name=f"tb{nm}")
                nc.sync.dma_start(out=t[:, :, :],
                                  in_=d.ap().rearrange("p (m d) -> p m d", d=HD))
                for m in range(MT):
                    tabs[(nm, m)] = t[:, m, :]

            # qkv weights in 4 grouped DMAs of 5 chunks each (sync queue)
            wq_g = []
            for g in range(4):
                t = wqp.tile([128, 5, WQCOLS], f16, tag="wq", name="wq")
                nc.sync.dma_start(
                    out=t[:, :, :],
                    in_=wq_d.ap()[:, g * 5 * WQCOLS:(g + 1) * 5 * WQCOLS]
                    .rearrange("p (k c) -> p k c", c=WQCOLS))
                wq_g.append(t)
            wq_sb = [wq_g[kc // 5][:, kc % 5, :] for kc in range(KCH)]

            # o_proj weights in one DMA (gpsimd queue; needed late)
            wo_t = wop.tile([128, NH, OC], f16, tag="wo", name="wo")
            nc.gpsimd.dma_start(out=wo_t[:, :, :],
                                in_=wo_d.ap().rearrange("p (k c) -> p k c", c=OC))
            wo_sb = [wo_t[:, r, :] for r in range(NH)]

            qT3 = kvp.tile([128, 3, S], f16, tag="qT3", name="qT3")
            KT2 = kvp.tile([128, 2, S], f16, tag="KT2", name="KT2")
            VV = [[kvp.tile([128, 128], f16, tag=f"V{b}_{m}", name=f"V{b}_{m}")
                   for m in range(MT)] for b in range(2)]

            def norm_rope_batched(eng, xn_view, t1, t2, ob_view, scratch_tag):
                """xn_view [128, nh, 128] normalized input; tables [128, 128];
                writes roped fp16 into ob_view [128, nh, 128]."""
                nh = xn_view.shape[1]
                se = xn_view.rearrange("p h (i two) -> p h i two", two=2)
                t1b = t1.rearrange("p (one d) -> p one d", one=1).to_broadcast(
                    [128, nh, HD])
                t2b = t2.rearrange("p (one d) -> p one d", one=1).to_broadcast(
                    [128, nh, HD])
                t1e = t1b.rearrange("p h (i two) -> p h i two", two=2)
                t2e = t2b.rearrange("p h (i two) -> p h i two", two=2)
                ob = ob_view.rearrange("p h (i two) -> p h i two", two=2)
                a1 = nrp.tile([128, nh, 64], f32, tag=f"ra1{scratch_tag}",
                              name=f"ra1{scratch_tag}")
                a2 = nrp.tile([128, nh, 64], f32, tag=f"ra2{scratch_tag}",
                              name=f"ra2{scratch_tag}")
                eng.tensor_mul(a1[:, :, :], se[:, :, :, 0], t1e[:, :, :, 0])
                eng.tensor_mul(a2[:, :, :], se[:, :, :, 1], t2e[:, :, :, 1])
                eng.tensor_sub(ob[:, :, :, 0], a1[:, :, :], a2[:, :, :])
                eng.tensor_mul(a1[:, :, :], se[:, :, :, 0], t2e[:, :, :, 0])
                eng.tensor_mul(a2[:, :, :], se[:, :, :, 1], t1e[:, :, :, 1])
                eng.tensor_add(ob[:, :, :, 1], a1[:, :, :], a2[:, :, :])

            def emit_front(m):
                """Quantize x tile, PE-transpose into qm [128, KCH, 128]."""
                xa = xa_tiles.pop(m)
                q16 = q16p.tile([128, DIM], i16, tag="q16", name="q16")
                nc.scalar.activation(q16[:, :], xa[:, :],
                                     mybir.ActivationFunctionType.Copy,
                                     scale=s_cols[:, m:m + 1])
                q16f = qfp.tile([128, DIM], f16, tag="q16f", name="q16f")
                nc.vector.tensor_copy(q16f[:, :], q16[:, :])
                qm = qmp.tile([128, KCH, 128], f16, tag="qm", name="qm")
                qmf = qm.rearrange("p k d -> p (k d)")
                for g in range(3):
                    nch = 8 if g < 2 else 4
                    ps = trp.tile([128, 1024], f16, tag="tr", name="tr")
                    for j in range(nch):
                        kc = 8 * g + j
                        nc.tensor.transpose(ps[:, j * 128:(j + 1) * 128],
                                            q16f[:, kc * 128:(kc + 1) * 128],
                                            ident_h[:, :])
                    if g == 0:
                        nc.vector.tensor_copy(
                            qmf[:, g * 1024:g * 1024 + nch * 128],
                            ps[:, 0:nch * 128])
                    else:
                        nc.scalar.copy(qmf[:, g * 1024:g * 1024 + nch * 128],
                                       ps[:, 0:nch * 128])
                return qm

            def emit_qkv(m, qm):
                """QKV matmul for token tile m + rms/rope epilogue -> rbq."""
                psA = cmm.tile([128, 384], f32, tag="cmm", name="cA")
                psB = cmm.tile([128, 512], f32, tag="cmm", name="cB")
                for kc in range(KCH):
                    lh = qm[:, kc, :]
                    nc.tensor.matmul(psA[:, :], lh, wq_sb[kc][:, 0:384],
                                     start=(kc == 0), stop=(kc == KCH - 1))
                    nc.tensor.matmul(psB[:, :], lh, wq_sb[kc][:, 384:896],
                                     start=(kc == 0), stop=(kc == KCH - 1))
                rs_ap = rs_cols[:, m:m + 1]
                qxs = nrp.tile([128, 384], f32, tag="qxs", name="qxs")
                nc.scalar.copy(qxs[:, :], psA[:, :])
                kxs = nrp.tile([128, 2, 128], f32, tag="kxs", name="kxs")
                nc.scalar.copy(kxs[:, :, :],
                               psB.rearrange("p (b c) -> p b c", c=256)[:, :, 0:128])
                for blk in range(2):
                    nc.scalar.activation(VV[blk][m][:, :],
                                         psB[:, blk * 256 + 128:blk * 256 + 256],
                                         mybir.ActivationFunctionType.Copy,
                                         scale=rs_ap)
                # rms factors: q-path on DVE, k-path on GpSimd
                rs5 = nrp.tile([128, 5], f32, tag="rs5", name="rs5")
                sq = nrp.tile([128, 384], f32, tag="sqq", name="sqq")
                nc.vector.tensor_mul(sq[:, :], qxs[:, :], qxs[:, :])
                nc.vector.tensor_reduce(rs5[:, 0:3],
                                        sq.rearrange("p (h d) -> p h d", d=128),
                                        mybir.AxisListType.X, mybir.AluOpType.add)
                nc.vector.tensor_scalar(rs5[:, 0:3], rs5[:, 0:3], 1.0 / HD, EPS,
                                        mybir.AluOpType.mult, mybir.AluOpType.add)
                sk = nrp.tile([128, 256], f32, tag="sqk", name="sqk")
                nc.gpsimd.tensor_mul(sk[:, :], kxs.rearrange("p b c -> p (b c)"),
                                     kxs.rearrange("p b c -> p (b c)"))
                nc.vector.tensor_reduce(rs5[:, 3:5],
                                        sk.rearrange("p (h d) -> p h d", d=128),
                                        mybir.AxisListType.X, mybir.AluOpType.add)
                nc.vector.tensor_scalar(rs5[:, 3:5], rs5[:, 3:5], 1.0 / HD, EPS,
                                        mybir.AluOpType.mult, mybir.AluOpType.add)
                nc.vector.reciprocal_approx_fast(rs5[:, :], rs5[:, :])
                nc.scalar.activation(rs5[:, :], rs5[:, :],
                                     mybir.ActivationFunctionType.Sqrt)
                for h in range(3):
                    nc.vector.tensor_scalar_mul(qxs[:, h * 128:(h + 1) * 128],
                                                qxs[:, h * 128:(h + 1) * 128],
                                                rs5[:, h:h + 1])
                for h in range(2):
                    nc.vector.tensor_scalar_mul(kxs[:, h, :], kxs[:, h, :],
                                                rs5[:, 3 + h:4 + h])
                rbq = rbp.tile([128, 5, HD], f16, tag="rbq", name="rbq")
                norm_rope_batched(nc.gpsimd,
                                  qxs.rearrange("p (h d) -> p h d", d=128),
                                  tabs[("tq1", m)], tabs[("tq2", m)],
                                  rbq[:, 0:3, :], "q")
                norm_rope_batched(nc.gpsimd, kxs[:, :, :],
                                  tabs[("tk1", m)], tabs[("tk2", m)],
                                  rbq[:, 3:5, :], "k")
                return rbq

            def emit_ropeT(m, rbq):
                """PE-transpose roped q/k of tile m into qT3 / KT2 columns."""
                ps = trp.tile([128, 1024], f16, tag="tr", name="tr")
                for sl in range(5):
                    nc.tensor.transpose(ps[:, sl * 128:(sl + 1) * 128],
                                        rbq[:, sl, :], ident_h[:, :])
                psv = ps.rearrange("p (s d) -> p s d", d=128)
                nc.scalar.copy(qT3[:, :, m * 128:(m + 1) * 128], psv[:, 0:3, :])
                nc.scalar.copy(KT2[:, :, m * 128:(m + 1) * 128], psv[:, 3:5, :])

            def emit_attn(t):
                """Attention for q quarter t (tokens t*256..t*256+255), 3 slots,
                then pack fp16 (attn/den) into agin and fire the AllGather."""
                qs = slice(t * QW, (t + 1) * QW)
                npair = t + 1
                sl_state = []
                deferred = []
                for sl in range(3):
                    blk = 0 if sl < 2 else 1
                    cs = csp.tile([1, QW], f32, tag="cs", name="cs")
                    av = avpp.tile([128, QW], f32, tag="av", name="av")
                    for j in range(npair):
                        ps = scp.tile([128, 2, QW], f32, tag="sc", name="sc")
                        for b in range(2):
                            kc = 2 * j + b
                            nc.tensor.matmul(ps[:, b, :],
                                             KT2[:, blk, kc * 128:(kc + 1) * 128],
                                             qT3[:, sl, qs], start=True, stop=True)
                        pt = ptp.tile([128, 2, QW], f16, tag="pt", name="pt")
                        ptf = pt.rearrange("p b q -> p (b q)")
                        nc.scalar.activation(ptf[:, :],
                                             ps.rearrange("p b q -> p (b q)")[:, :],
                                             mybir.ActivationFunctionType.Exp,
                                             bias=eshift[:, 0:1], scale=SCALE)
                        if j == npair - 1:  # diagonal pair
                            nc.vector.tensor_mul(ptf[:, :], ptf[:, :], cmask[:, :])
                        for b in range(2):
                            kc = 2 * j + b
                            nc.tensor.matmul(cs[0:1, :], ones_c16[:, 0:1],
                                             pt[:, b, :], start=(kc == 0),
                                             stop=(kc == 2 * npair - 1))
                            nc.tensor.matmul(av[:, :], VV[blk][kc][:, :],
                                             pt[:, b, :], start=(kc == 0),
                                             stop=(kc == 2 * npair - 1))
                    rden = atp.tile([1, QW], f32, tag="rden", name="rden")
                    nc.vector.reciprocal_approx_fast(rden[:, :], cs[:, :])
                    sl_state.append((av, rden))
                    # emit the previous slot's normalize/pack now, so its PE
                    # broadcast never stalls on this slot's DVE chain
                    if sl >= 1:
                        deferred.append(emit_pack(t, sl - 1, *sl_state[sl - 1]))
                deferred.append(emit_pack(t, 2, *sl_state[2]))
                nc.gpsimd.collective_compute(
                    "AllGather", mybir.AluOpType.bypass,
                    ins=[agin_d[t].ap().opt()], outs=[agout_d[t].ap().opt()],
                    replica_groups=[list(range(NC))],
                )
                # gathered tile: single DMA, lands as the collective completes
                agt = agtp.tile([128, 24, QW], f16, tag="agt", name="agt")
                nc.scalar.dma_start(
                    out=agt[:, :, :],
                    in_=agout_d[t].ap().rearrange("(c p) q -> p c q", p=128))
                return agt

            def emit_pack(t, sl, av, rden):
                bf = trp.tile([128, QW], f32, tag="tr", name="bf")
                nc.tensor.matmul(bf[:, :], ones_r32[0:1, :], rden[0:1, :],
                                 start=True, stop=True)
                fac = atp.tile([128, QW], f32, tag="fac", name="fac")
                nc.scalar.copy(fac[:, :], bf[:, :])
                aq = atp.tile([128, QW], f16, tag="aq", name="aq")
                nc.vector.tensor_mul(aq[:, :], av[:, :], fac[:, :])
                nc.sync.dma_start(out=agin_d[t][sl * 128:(sl + 1) * 128, :],
                                  in_=aq[:, :])
                return aq

            def emit_oproj(t, agt):
                """o_proj for q quarter t from gathered attention outs."""
                for lo, hi in ((0, 128), (128, 256), (256, 320)):
                    pw = hi - lo
                    pso = cmm.tile([128, QW], f32, tag="cmm", name="pso")
                    for r in range(NH):
                        nc.tensor.matmul(pso[0:pw, :], wo_sb[r][:, lo:hi],
                                         agt[:, REAL_CHUNKS[r], :],
                                         start=(r == 0), stop=(r == NH - 1))
                    oev = oevp.tile([128, QW], f32, tag="oev", name="oev")
                    nc.scalar.copy(oev[0:pw, :], pso[0:pw, :])
                    nc.sync.dma_start(out=out_d[lo:hi, t * QW:(t + 1) * QW],
                                      in_=oev[0:pw, :])

            # emission order: ropeT(m-1) is deferred one tile so the PE never
            # waits on tile m-1's rms/rope chain; the next pair's front and
            # qkv precede attention of the finished pair for the same reason
            agts = {}
            rbq_prev = None
            for m in range(MT):
                qm = emit_front(m)
                if m + 2 < MT:
                    emit_xload(m + 2)
                rbq = emit_qkv(m, qm)
                if rbq_prev is not None:
                    emit_ropeT(m - 1, rbq_prev)
                rbq_prev = rbq
                if m % 2 == 0 and m >= 2:
                    agts[m // 2 - 1] = emit_attn(m // 2 - 1)
                if m == 6:
                    emit_oproj(0, agts.pop(0))
                if m == 7:
                    emit_oproj(1, agts.pop(1))
            emit_ropeT(MT - 1, rbq_prev)
            agts[3] = emit_attn(3)
            emit_oproj(2, agts.pop(2))
            emit_oproj(3, agts.pop(3))

    nc.compile()
    return nc


def _host_prep(x, w_qkv, ws_qkv, w_o, ws_o, q_norm_w, k_norm_w):
    w_dq = (w_qkv * np.repeat(ws_qkv, GS, axis=1)).astype(np.float32)
    wo_dq = (w_o * np.repeat(ws_o, GS, axis=1)).astype(np.float32)

    pos = np.arange(S, dtype=np.float32)
    inv_freq = (THETA ** (-np.arange(0, HD, 2, dtype=np.float32) / HD)).astype(np.float32)
    ang = pos[:, None] * inv_freq[None, :]
    ce = np.repeat(np.cos(ang).astype(np.float32), 2, axis=1)
    se = np.repeat(np.sin(ang).astype(np.float32), 2, axis=1)

    def _pmajor(a, blk):
        # [N*128, C] -> [128, N*C] partition-major for cheap DMA descriptors
        n = a.shape[0] // 128
        return np.ascontiguousarray(
            a.reshape(n, 128, blk).transpose(1, 0, 2).reshape(128, n * blk))

    tq1 = _pmajor((ce * q_norm_w[None, :]).astype(np.float32), HD)
    tq2 = _pmajor((se * q_norm_w[None, :]).astype(np.float32), HD)
    tk1 = _pmajor((ce * k_norm_w[None, :]).astype(np.float32), HD)
    tk2 = _pmajor((se * k_norm_w[None, :]).astype(np.float32), HD)

    # diagonal-pair mask: scoresT [k(128), (kc_bit, q_tile) x 256]
    tri = np.triu(np.ones((128, 128), np.float32))  # keep k <= q
    cm = np.concatenate(
        [tri, np.ones((128, 128), np.float32),
         np.zeros((128, 128), np.float32), tri], axis=1)
    cmask = cm.astype(FP16)

    in_maps = []
    for c in range(NC):
        wq = np.zeros((DIM, WQCOLS), np.float32)
        for sl in range(3):
            h = HEADS[c][sl]
            if h is not None:
                wq[:, sl * 128:(sl + 1) * 128] = w_dq[h * HD:(h + 1) * HD, :].T
        ga = GA[c]
        wq[:, 384:512] = w_dq[KBASE + ga * HD:KBASE + (ga + 1) * HD, :].T
        wq[:, 512:640] = w_dq[VBASE + ga * HD:VBASE + (ga + 1) * HD, :].T
        gb = GB[c]
        if gb is not None:
            wq[:, 640:768] = w_dq[KBASE + gb * HD:KBASE + (gb + 1) * HD, :].T
            wq[:, 768:896] = w_dq[VBASE + gb * HD:VBASE + (gb + 1) * HD, :].T

        wo = np.zeros((NH * 128, OC), np.float32)
        for r, ck in enumerate(REAL_CHUNKS):
            j, sl = ck // 3, ck % 3
            h = HEADS[j][sl]
            wo[r * 128:(r + 1) * 128, :] = \
                wo_dq[c * OC:(c + 1) * OC, h * HD:(h + 1) * HD].T

        in_maps.append({
            "x": x.astype(np.float32),
            "wq": _pmajor(wq.astype(FP16), WQCOLS),
            "wo": _pmajor(wo.astype(FP16), OC),
            "tq1": tq1, "tq2": tq2, "tk1": tk1, "tk2": tk2,
            "cmask": cmask,
        })
    return in_maps


def kernel(x, w_qkv, ws_qkv, w_o, ws_o, q_norm_w, k_norm_w):
    x = np.asarray(x, np.float32)
    w_qkv = np.asarray(w_qkv, np.float32)
    ws_qkv = np.asarray(ws_qkv, np.float32)
    w_o = np.asarray(w_o, np.float32)
    ws_o = np.asarray(ws_o, np.float32)
    q_norm_w = np.asarray(q_norm_w, np.float32)
    k_norm_w = np.asarray(k_norm_w, np.float32)

    if "nc" not in _cached:
        _cached["nc"] = _build_nc()
    nc = _cached["nc"]

    in_maps = _host_prep(x, w_qkv, ws_qkv, w_o, ws_o, q_norm_w, k_norm_w)
    trace = bool(int(os.environ.get("BENCH_TRACE", "0")))
    res = run_bass_kernel_spmd(nc, in_maps, core_ids=list(range(NC)), trace=trace)
    if trace and res.exec_time_ns is not None:
        print(f"HW exec time: {res.exec_time_ns} ns")
        _cached["exec_time_ns"] = res.exec_time_ns

    # per-core output is [OC, S]; stack along features then transpose
    out = np.concatenate([np.asarray(res.results[c]["out"], np.float32)
                          for c in range(NC)], axis=0).T.copy()
    return out
